# revision 44
# baseline (speedup 1.0000x reference)
"""AttentionReadout kernel for Trainium2 (8 NeuronCores, Bass/Tile), v2.

Math (reference):
    feat_u = feat @ W_u.T                           [N, D]
    feat_v = feat[last_nodes] @ W_v.T + b_v         [B, D]
    e      = sigmoid(feat_u + feat_v[segment_ids]) @ w_e   [N]
    alpha  = e * cnt                                [N]
    rst    = segment_sum(feat * alpha[:, None], segment_ids, B)   [B, D]

v2 strategy (per core, 256 segments, nodes packed into per-segment column
slots; one shared SPMD program, all shapes from the cross-core max slot
widths):
  - z-path: fp8(e4m3) DoubleRow matmuls: lhsT = Wu chunks [128,2,128],
    rhs = feat in transposed fp8 layout fdr [128,2,cols]; K=256 in one
    0.5-cyc/col pass.  feat_v bias is PRE-FILLED into the psum bank via a
    rank-1 fp8 DoubleRow matmul (stationary = the segment's feat_v row,
    moving = ones), so the sigmoid needs no per-segment bias.
  - sigmoid: one ACT instruction per psum BANK; segments are FFD-packed
    into 512-col banks (usually 2 segs/bank) -> ~130 insts instead of 512.
  - e per node: matmul with sig [128feat, 128cols] as STATIONARY and
    w_e chunk [128,1] as moving -> e lands node-partitioned in psum,
    1 column per 128 nodes (virtually free on PE).
  - readout: alpha-selector matmul.  sel[n, j] = cnt_n * e_n * mask where
    mask (host-built, bf16) marks which of the <=3 segments in this
    128-node window node n belongs to.  matmul(lhsT=sel [128,3],
    rhs = natural bf16 feat rows [128,256]) accumulates rst rows directly
    in psum.  This removes the old DVE scalar_tensor_tensor readout
    (167us) and the alpha TensorTensor (84us) entirely.
  - cnt is folded into the host-built mask; cnt_rep is no longer shipped.
"""

import math
from contextlib import ExitStack

import numpy as np
import ml_dtypes

import concourse.bass as bass
import concourse.mybir as mybir
import concourse.tile as tile
from concourse.bass_utils import run_bass_kernel_spmd

BF16NP = ml_dtypes.bfloat16
FP8NP = ml_dtypes.float8_e4m3
F32 = mybir.dt.float32
BF16 = mybir.dt.bfloat16
FP8 = mybir.dt.float8e4
AFT = mybir.ActivationFunctionType
OP = mybir.AluOpType
DRM = mybir.MatmulPerfMode.DoubleRow

N_CORES = 8
D = 256
B = 2048
NSEG = B // N_CORES     # 256 segments per core
KC = D // 128           # feature chunks
BANK = 512              # psum f32 cols per bank
SW = 3                  # selector window (max segments per 128-node chunk)
BATCH_COLS = 5120       # target batch fill before 128-align padding


_SPLITTABLE = {
    "InstActivation", "InstMatmult", "InstLdweights", "InstTensorTensor",
    "InstTensorScalarPtr", "InstTensorCopy", "InstMemset", "InstNoOp",
    "InstTensorReduce", "InstCopyPredicated", "InstIota", "InstDrain",
    "InstDMACopy",
}


def _split_multi_waits(nc):
    """Walrus accepts one sync-wait per instruction; split extras to NoOps."""
    n = 0
    for f in nc.m.functions:
        for blk in f.blocks:
            insts = blk.instructions
            i = 0
            while i < len(insts):
                inst = insts[i]
                si = inst.sync_info
                if si is None or inst.__class__.__name__ not in _SPLITTABLE \
                        or len(si.on_wait) <= 1:
                    i += 1
                    continue
                merged, rest = {}, []
                for w in si.on_wait:
                    if (w.sync_type == "semaphore" and w.wait_mode == "sem-ge-imm"
                            and w.wait_reg is None):
                        if w.id not in merged or w.wait_value > merged[w.id].wait_value:
                            merged[w.id] = w
                    else:
                        rest.append(w)
                waits = list(merged.values()) + rest
                inst.sync_info = mybir.SyncInfo(
                    on_wait=[waits[-1]], on_update=list(si.on_update))
                for w in waits[:-1]:
                    n += 1
                    nop = mybir.InstNoOp(
                        name=f"I-wsplit-{n}", bass_nofuse=True, engine=inst.engine,
                        sync_info=mybir.SyncInfo(on_wait=[w], on_update=[]))
                    insts.insert(i, nop)
                    i += 1
                i += 1
    return n


# ---------------------------------------------------------------- planning
class Plan:
    pass


def plan_layout(lens):
    """Shared (cross-core) column layout.

    Returns Plan with:
      perms0 [8, 256]: core's rank r -> local segment id (sorted desc)
      rank_of_oidx [256]: column-order position -> rank
      slot_w [256 in oidx order], col_off [256], total_cols (128-mult)
      mbs: list of (oidx list, width list, mb_cols, pad) per bank
      batches: list of dicts {c0, W, t0, nch, mbs: [...]}
      chunk_base [CH]: (sc, base) for readout window
    """
    per_core = lens.reshape(N_CORES, NSEG)
    perms0 = np.argsort(-per_core, axis=1, kind="stable")
    sorted_lens = np.take_along_axis(per_core, perms0, axis=1)
    widths = sorted_lens.max(axis=0)                      # [256] desc
    slots = np.maximum(16, widths.astype(np.int64))
    if slots.min() < 64 or slots.max() > BANK:
        return None

    # FFD-pack ranks into <=512-col psum banks
    bins = []           # [remaining, [ranks]]
    for r in range(NSEG):
        w = slots[r]
        for bn in bins:
            if bn[0] >= w:
                bn[0] -= w
                bn[1].append(r)
                break
        else:
            bins.append([BANK - w, [r]])

    p = Plan()
    p.perms0 = perms0
    rank_of_oidx = []
    col = 0
    batches = []
    bi = 0
    est_total = int(slots.sum())
    while bi < len(bins):
        batch = {"c0": col, "mbs": []}
        tgt = BATCH_COLS

        def _take_bin():
            nonlocal col, bi
            ranks = bins[bi][1]
            ws = [int(slots[r]) for r in ranks]
            batch["mbs"].append({
                "oidx": list(range(len(rank_of_oidx),
                                   len(rank_of_oidx) + len(ranks))),
                "w": ws, "W": sum(ws), "pad": 0})
            rank_of_oidx.extend(ranks)
            col += sum(ws)
            bi += 1

        while bi < len(bins) and (col - batch["c0"]) < tgt:
            _take_bin()
        # keep the per-batch psum-tile count EVEN (incl. the pad mb): with
        # bufs=2 pz tiles, an odd count makes the next batch's first
        # z-matmul recycle the bank of the PREVIOUS batch's LAST sigmoid,
        # fully serializing the batch boundary.
        pad = (-col) % 128
        while bi < len(bins) and (len(batch["mbs"]) + (1 if pad else 0)) % 2:
            _take_bin()
            pad = (-col) % 128
        if pad:
            batch["mbs"].append({"oidx": [], "w": [], "W": pad, "pad": pad})
            col += pad
        batch["W"] = col - batch["c0"]
        batches.append(batch)
    p.rank_of_oidx = np.array(rank_of_oidx)
    p.slot_w = slots[p.rank_of_oidx]                      # width per oidx
    p.col_off = np.zeros(NSEG, np.int64)                  # per oidx
    p.total_cols = col
    # recompute offsets per oidx by walking batches
    off = {}
    c = 0
    for b in batches:
        c = b["c0"]
        for mb in b["mbs"]:
            for o, w in zip(mb["oidx"], mb["w"]):
                off[o] = c
                c += w
            c += mb["pad"]
    for o, v in off.items():
        p.col_off[o] = v
    for b in batches:
        b["t0"] = b["c0"] // 128
        b["nch"] = b["W"] // 128
    p.batches = batches

    # chunk -> (sc, base oidx of window)
    CH = p.total_cols // 128
    oidx_of_col = np.full(p.total_cols, -1, np.int64)
    for o in range(NSEG):
        oidx_of_col[p.col_off[o]: p.col_off[o] + p.slot_w[o]] = o
    p.oidx_of_col = oidx_of_col
    p.chunk_base = []
    for t in range(CH):
        win = oidx_of_col[128 * t: 128 * (t + 1)]
        valid = win[win >= 0]
        if valid.size == 0:
            p.chunk_base.append(0)
            continue
        base = min(int(valid.min()), NSEG - SW)
        if valid.max() >= base + SW:
            return None            # window wider than SW; bail to fallback
        p.chunk_base.append(base)
    p.CH = CH
    return p


# ---------------------------------------------------------------- device code
def build_program(p, split_waits=True):
    nc = bass.Bass()
    NPP = p.total_cols
    CH = p.CH

    fdr = nc.dram_tensor("fdr", [128, KC, NPP], FP8, kind="ExternalInput")
    fnat = nc.dram_tensor("fnat", [128, CH, D], BF16, kind="ExternalInput")
    msk = nc.dram_tensor("msk", [128, CH, SW], BF16, kind="ExternalInput")
    wudr = nc.dram_tensor("wudr", [KC, 128, KC, 128], FP8, kind="ExternalInput")
    fvdr = nc.dram_tensor("fvdr", [NSEG, KC, 2, 128], FP8, kind="ExternalInput")
    wec = nc.dram_tensor("wec", [KC, 128, 1], BF16, kind="ExternalInput")
    onesd = nc.dram_tensor("onesd", [128, KC, BANK], FP8, kind="ExternalInput")
    rstp_out = nc.dram_tensor("rstp", [128, KC, NSEG], F32, kind="ExternalOutput")

    with tile.TileContext(nc) as tc, ExitStack() as ctx:
        const = ctx.enter_context(tc.tile_pool(name="const", bufs=1))
        wudr_c = const.tile([128, KC, KC, 128], FP8, tag="wudr", name="wudr_c")
        wec_c = const.tile([128, KC, 1], BF16, tag="wec", name="wec_c")
        ones_t = const.tile([128, KC, BANK], FP8, tag="ones", name="ones")
        wudr_t = [wudr_c[:, m, :, :] for m in range(KC)]
        wec_t = [wec_c[:, m, :] for m in range(KC)]

        # one DMA per const, dispatched from the ACT queue so they don't
        # serialize behind the batch-0 loads on the SP sequencer
        nc.scalar.dma_start(wudr_c[:], wudr[:].rearrange("m p i q -> p m i q"))
        nc.scalar.dma_start(ones_t[:], onesd[:])

        # persistent psum: rst rows + e columns
        prst = ctx.enter_context(tc.tile_pool(name="prst", bufs=1, space="PSUM"))
        rst_ps = prst.tile([128, KC, NSEG], F32, tag="rst", name="rst_ps")      # 1 bank
        pec = ctx.enter_context(tc.tile_pool(name="pec", bufs=3, space="PSUM"))
        nc.vector.memset(rst_ps[:], 0.0)

        pz = ctx.enter_context(tc.tile_pool(name="pz", bufs=2, space="PSUM"))
        fvp = ctx.enter_context(tc.tile_pool(name="fvp", bufs=2))
        fpool = ctx.enter_context(tc.tile_pool(name="fpool", bufs=2))
        npool = ctx.enter_context(tc.tile_pool(name="npool", bufs=3))
        mpool = ctx.enter_context(tc.tile_pool(name="mpool", bufs=3))
        spool = ctx.enter_context(tc.tile_pool(name="spool", bufs=3))
        selp = ctx.enter_context(tc.tile_pool(name="selp", bufs=2))

        def emit_chunk_range(b, stile, ntile, mtile, ecol_ps, ta, tb, part):
            t0, nch = b["t0"], b["nch"]
            if tb <= ta:
                return
            for t in range(ta, tb):
                co = 128 * (t - t0)
                for m in range(KC):
                    nc.tensor.matmul(ecol_ps[:, t - t0:t - t0 + 1],
                                     stile[:, m, co:co + 128], wec_t[m][:],
                                     start=(m == 0), stop=(m == KC - 1))
            nw = tb - ta
            sel = selp.tile([128, nw, SW], BF16, tag=f"sel{part}",
                            name="sel")
            nc.vector.tensor_tensor(
                out=sel[:], in0=mtile[:, ta - t0:tb - t0, :],
                in1=ecol_ps[:, ta - t0:tb - t0]
                    .rearrange("p (c o) -> p c o", o=1)
                    .broadcast_to([128, nw, SW]),
                op=OP.mult)
            for t in range(ta, tb):
                gbase = p.chunk_base[t]
                for m in range(KC):
                    nc.tensor.matmul(
                        rst_ps[:, m, gbase:gbase + SW],
                        ntile[:, t - t0, m * 128:(m + 1) * 128],
                        sel[:, t - ta, :],
                        start=False, stop=True, skip_group_check=True)

        def issue_readout_loads(b):
            # fnat/msk feed only the (two-batch-delayed) chunk phase; issuing
            # them AFTER the next batch's z-loads keeps the serial DMA queue
            # from stalling the z -> sigmoid critical chain.
            t0, nch = b["t0"], b["nch"]
            ntile = npool.tile([128, nch, D], BF16, tag="fnat", name="ntile")
            for q0 in range(0, nch, 9):
                q1 = min(q0 + 9, nch)
                nc.sync.dma_start(ntile[:, q0:q1, :],
                                  fnat[:, t0 + q0:t0 + q1, :])
            mtile = mpool.tile([128, nch, SW], BF16, tag="msk", name="mtile")
            nc.sync.dma_start(mtile[:], msk[:, t0:t0 + nch, :])
            return ntile, mtile

        pending = []    # [(b, stile, ntile, mtile), ...] awaiting chunk phase
        wec_loaded = [False]
        for b in p.batches:
            c0, W, t0, nch = b["c0"], b["W"], b["t0"], b["nch"]
            o_lo = min((mb["oidx"][0] for mb in b["mbs"] if mb["oidx"]),
                       default=0)
            o_hi = max((mb["oidx"][-1] + 1 for mb in b["mbs"] if mb["oidx"]),
                       default=1)
            fvb = fvp.tile([1, o_hi - o_lo, KC, 2, 128], FP8, tag="fvb",
                           name="fvb")
            nc.sync.dma_start(fvb[:], fvdr[o_lo:o_hi])
            ftile = fpool.tile([128, KC, W], FP8, tag="fdr", name="ftile")
            # split the load so early z-matmuls start after the first piece
            for q0 in range(0, W, 1088):
                q1 = min(q0 + 1088, W)
                nc.sync.dma_start(ftile[:, :, q0:q1],
                                  fdr[:, :, c0 + q0:c0 + q1])
            stile = spool.tile([128, KC, W], BF16, tag="sig", name="stile")
            if not wec_loaded[0]:
                wec_loaded[0] = True
                nc.scalar.dma_start(wec_c[:], wec[:].rearrange("m p o -> p m o"))

            lo = 0
            for mbi, mb in enumerate(b["mbs"]):
                if mbi == 6 and len(pending) > 1:
                    emit_chunk_range(*pending.pop(0))
                Wmb = mb["W"]
                pzt = pz.tile([128, KC, BANK], F32, tag="pz", name="pzt")
                for m in range(KC):
                    o = 0
                    for oidx, w in zip(mb["oidx"], mb["w"]):
                        nc.tensor.matmul(
                            pzt[:, m, o:o + w],
                            fvb[0:1, oidx - o_lo, m, :, :],
                            ones_t[0:1, :, 0:w],
                            start=True, stop=False, perf_mode=DRM,
                            skip_group_check=True)
                        nc.tensor.matmul(
                            pzt[:, m, o:o + w], wudr_t[m][:],
                            ftile[:, :, lo + o:lo + o + w],
                            start=False, stop=True, perf_mode=DRM,
                            skip_group_check=True)
                        o += w
                    if mb["pad"]:
                        nc.tensor.matmul(
                            pzt[:, m, o:o + mb["pad"]],
                            fvb[0:1, 0, m, :, :], ones_t[0:1, :, 0:mb["pad"]],
                            start=True, stop=True, perf_mode=DRM,
                            skip_group_check=True)
                nc.scalar.activation(stile[:, :, lo:lo + Wmb],
                                     pzt[:, :, 0:Wmb], AFT.Sigmoid)
                lo += Wmb

            ntile, mtile = issue_readout_loads(b)
            # chunks touching only EARLY mbs process right away (they need
            # only early sigmoids, so they overlap this batch's ACT tail);
            # the last-2-mb chunks defer into the next batch's mb loop so
            # they never sit in front of its z-matmuls on the in-order PE.
            ec = pec.tile([128, BANK], F32, tag="ecol", name="ecol_ps")
            wlast2 = sum(mb["W"] for mb in b["mbs"][-2:])
            t_split = t0 + max(0, (W - wlast2) // 128)
            emit_chunk_range(b, stile, ntile, mtile, ec, t0, t_split, "a")
            pending.append((b, stile, ntile, mtile, ec,
                            t_split, t0 + nch, "b"))
        # rows for order positions below the last batch's window are final
        # once its chunk phases are emitted; ship them early so only the
        # last batch's rows sit in the end-of-kernel chain.
        # progressive output: pending tails only REALLY write rows >= their
        # first window base (spill columns below carry zero mask), so after
        # each tail drains, the rows it finalized ship immediately.
        rst_sb = const.tile([128, KC, NSEG], F32, tag="rstsb", name="rst_sb")

        def ship_rows(a, bnd):
            if bnd > a:
                nc.scalar.activation(rst_sb[:, :, a:bnd],
                                     rst_ps[:, :, a:bnd], AFT.Identity)
                nc.sync.dma_start(rstp_out[:, :, a:bnd], rst_sb[:, :, a:bnd])

        shipped = 0
        while pending:
            cut = min(min(p.chunk_base[t] for t in range(ta2, tb2))
                      for (_, _, _, _, _, ta2, tb2, _) in pending)
            ship_rows(shipped, cut)
            shipped = max(shipped, cut)
            emit_chunk_range(*pending.pop(0))
        ship_rows(shipped, NSEG)

    if split_waits:
        _split_multi_waits(nc)
    return nc


# ---------------------------------------------------------------- host prep
def host_prep(feat, cnt, bounds, p):
    feat8 = feat.astype(FP8NP)
    feat16 = feat.astype(BF16NP)
    cnt16 = cnt.astype(BF16NP)
    NPP, CH = p.total_cols, p.CH

    in_maps = []
    for c in range(N_CORES):
        s0 = c * NSEG
        node_of_col = np.full(NPP, -1, np.int64)
        for o in range(NSEG):
            rank = p.rank_of_oidx[o]
            seg = p.perms0[c][rank]
            ln = int(bounds[s0 + seg + 1] - bounds[s0 + seg])
            ln = min(ln, int(p.slot_w[o]))
            node_of_col[p.col_off[o]:p.col_off[o] + ln] = bounds[s0 + seg] + \
                np.arange(ln)
        valid = node_of_col >= 0
        nodes = node_of_col[valid]

        fdr = np.zeros((128, KC, NPP), FP8NP)
        fdr[:, :, valid] = feat8[nodes].reshape(-1, KC, 128).transpose(2, 1, 0)

        nvc = node_of_col.reshape(CH, 128)
        vv = nvc >= 0
        fnat = feat16[nvc.clip(0)]            # [CH, 128, D]
        fnat[~vv] = 0
        fnat = np.ascontiguousarray(fnat.transpose(1, 0, 2))   # [128, CH, D]

        ovc = p.oidx_of_col.reshape(CH, 128)
        mask = np.zeros((CH, 128, SW), BF16NP)
        cw = cnt16[nvc.clip(0)]
        cw[~vv] = 0
        for j in range(SW):
            basej = np.array([p.chunk_base[t] + j
                              for t in range(CH)])[:, None]
            mask[:, :, j] = np.where(ovc == basej, cw, 0)
        mask = np.ascontiguousarray(mask.transpose(1, 0, 2))   # [128, CH, SW]

        in_maps.append({"fdr": fdr, "fnat": fnat, "msk": mask})
    return in_maps


def host_const(W_u, w_e):
    # wudr[m][p, i, q] = W_u[m*128+q, i*128+p]
    wu8 = W_u.astype(FP8NP)
    wudr = np.ascontiguousarray(
        wu8.reshape(KC, 128, KC, 128).transpose(2, 3, 0, 1)  # [i, p, m, q]
        .transpose(2, 1, 0, 3))                               # [m, p, i, q]
    wecv = np.ascontiguousarray(w_e.astype(BF16NP).reshape(KC, 128, 1))
    ones = np.zeros((128, KC, BANK), FP8NP)
    ones[:, 0, :] = 1.0
    return wudr, wecv, ones


def assemble(results, p):
    out = np.empty((B, D), np.float32)
    for c, r in enumerate(results):
        rstp = r["rstp"]          # [128, KC, NSEG] = rst[seg, m*128+p]
        s0 = c * NSEG
        rows = rstp.transpose(2, 1, 0).reshape(NSEG, D)   # [oidx, D]
        segs = p.perms0[c][p.rank_of_oidx]
        out[s0 + segs] = rows
    return out


def _reference_numpy(feat, cnt, segment_ids, last_nodes, W_u, W_v, b_v, w_e):
    feat_u = feat @ W_u.T
    feat_v = feat[last_nodes] @ W_v.T + b_v
    z = feat_u + feat_v[segment_ids]
    e = (1.0 / (1.0 + np.exp(-z))) @ w_e
    alpha = (e * cnt).astype(np.float32)
    Bn = feat_v.shape[0]
    rst = np.zeros((Bn, feat.shape[1]), np.float32)
    np.add.at(rst, segment_ids, feat * alpha[:, None])
    return rst


_CACHE = {}
TRACE = False
LAST_RESULTS = None


def kernel(feat, cnt, segment_ids, last_nodes, W_u, W_v, b_v, w_e):
    feat = np.asarray(feat, np.float32)
    cnt = np.asarray(cnt, np.float32)
    segment_ids = np.asarray(segment_ids)
    last_nodes = np.asarray(last_nodes)
    N, d = feat.shape

    if (d != D or not np.all(np.diff(segment_ids) >= 0)
            or (segment_ids.size and int(segment_ids.max()) >= B)):
        return _reference_numpy(feat, cnt, segment_ids, last_nodes,
                                W_u, W_v, b_v, w_e)

    bounds = np.searchsorted(segment_ids, np.arange(B + 1)).astype(np.int64)
    lens = np.diff(bounds)
    p = plan_layout(lens)
    if p is None:
        return _reference_numpy(feat, cnt, segment_ids, last_nodes,
                                W_u, W_v, b_v, w_e)

    key = (tuple(p.slot_w), tuple(p.rank_of_oidx))
    if key not in _CACHE:
        _CACHE[key] = build_program(p)
    nc = _CACHE[key]

    wudr, wecv, ones = host_const(W_u, w_e)
    in_maps = host_prep(feat, cnt, bounds, p)
    # feat_v rows on host (bf16 inputs, f32 accum -> fp8), in oidx order
    fl16 = feat[last_nodes].astype(BF16NP).astype(np.float32)
    wv16 = W_v.astype(BF16NP).astype(np.float32)
    fv_all = (fl16 @ wv16.T + b_v).astype(FP8NP)       # [B, D]
    for c in range(N_CORES):
        s0 = c * NSEG
        segs = p.perms0[c][p.rank_of_oidx]            # local seg per oidx
        fv = fv_all[s0 + segs]                        # [256, D] in oidx order
        fvdr = np.zeros((NSEG, KC, 2, 128), FP8NP)
        fvdr[:, :, 0, :] = fv.reshape(NSEG, KC, 128)
        in_maps[c].update({"wudr": wudr, "fvdr": fvdr,
                           "wec": wecv, "onesd": ones})

    try:
        res = run_bass_kernel_spmd(nc, in_maps, core_ids=list(range(N_CORES)),
                                   trace=TRACE)
    except Exception as exc:
        import sys
        print(f"kernel: device path failed ({type(exc).__name__}: {exc}); "
              f"falling back to host computation", file=sys.stderr)
        return _reference_numpy(feat, cnt, segment_ids, last_nodes,
                                W_u, W_v, b_v, w_e)
    global LAST_RESULTS
    LAST_RESULTS = res
    return assemble(res.results, p)


if __name__ == "__main__":
    rng = np.random.default_rng(0)
    N = 200000
    feat = rng.standard_normal((N, D), dtype=np.float32)
    cnt = rng.random(N, dtype=np.float32)
    seg = np.sort(rng.integers(0, B, N).astype(np.int32))
    last = rng.integers(0, N, B).astype(np.int32)
    s = 1.0 / math.sqrt(D)
    W_u = rng.uniform(-s, s, (D, D)).astype(np.float32)
    W_v = rng.uniform(-s, s, (D, D)).astype(np.float32)
    b_v = rng.uniform(-s, s, D).astype(np.float32)
    w_e = rng.uniform(-s, s, D).astype(np.float32)
    out = kernel(feat, cnt, seg, last, W_u, W_v, b_v, w_e)
    exp = _reference_numpy(feat, cnt, seg, last, W_u, W_v, b_v, w_e)
    err = np.abs(out - exp).max() / (np.abs(exp).max() + 1e-9)
    print("rel err:", err)
